# revision 19
# baseline (speedup 1.0000x reference)
"""Trainium2 Bass kernel for nn_GroupQueryAttention_51616916963669.

GQA with YaRN RoPE, sliding-window (128) + causal mask, learned sink logit,
qkv/out projections. B=1, S=2048, E=2048, H=32, G=8, D=64.

Sharding over 8 NeuronCores: 2-way sequence (1024 queries each, with a
128-token KV halo) x 4-way heads (8 q-heads / 2 kv-groups each). Each core
computes a partial out-projection (over its 512 ctx dims); the host sums the
4 head-partials per sequence half and concatenates.

v5 design notes (transposed-scores attention):
- x streams in 8 chunked DMAs over all 4 HWDGE queues; the K/V projection
  runs e-outer so the PE consumes chunks as they land (no 16us DMA wait).
- Q projection skips the 128-token halo (only K/V need it).
- Scores are computed TRANSPOSED ([kv, q]): psum = K^T Q per kv-block with
  the triangular band mask added via an extra matmul (lhsT=mask^T,
  rhs=replicated identity) straight into the same PSUM bank, so exp (ACT)
  reads a fully-masked tile and no DVE masking / row-sum / transposes exist.
- V is extended with a ones-column so the ctx matmul yields the softmax
  denominator for free in psum row 64; exp(sink) is preloaded into that row
  by a unit-row matmul. 1/denom comes from one DVE reciprocal, is
  replicated across partitions by a ones-matmul, and the ctx normalize is
  two DVE + two GpSimd [64,128] multiplies writing bf16 ctxT directly.
- V^T tiles come from dma_start_transpose (off-engine).
- Out-projection is split per sequence-half and interleaved with the
  attention tail; outputs leave in 8 DMAs round-robined over 4 queues.
"""
import numpy as np

# ---- problem constants (hardcoded per contract) ----
B, S, E = 1, 2048, 2048
H, G, D = 32, 8, 64
SW = 128
ROPE_BASE = 10000.0
ORIG_CTX = 4096.0
YARN_SCALE = 2.0
BETA_FAST, BETA_SLOW = 32.0, 1.0

# ---- sharding constants ----
NCORES = 8
TOK = 1152           # local kv tokens (9 blocks of 128)
NQ = 1024            # local query tokens (kv blocks 1..8)
QH = 8               # q heads per core
KG = 2               # kv groups per core
FTOT = QH * D + 2 * KG * D   # 768, feature order [K, V, Q0..Q3]
NE = E // 128        # 16 e-chunks
SCALE = 1.0 / (D ** 0.5)
MASKVAL = -30.0      # additive mask logit (exp(-30) ~ 1e-13)

_compiled = None
DEBUG = False


def _build_bass():
    import concourse.bacc as bacc
    import concourse.tile as tile
    import concourse.mybir as mybir
    from concourse.masks import make_identity

    f32 = mybir.dt.float32
    bf16 = mybir.dt.bfloat16
    Exp = mybir.ActivationFunctionType.Exp
    Ident = mybir.ActivationFunctionType.Identity

    nc = bacc.Bacc("TRN2", target_bir_lowering=False, debug=False,
                   num_devices=NCORES)

    xT = nc.dram_tensor("xT", [128, NE, TOK], bf16, kind="ExternalInput").ap()
    wqkvT = nc.dram_tensor("wqkvT", [128, 3, NE, 256], bf16,
                           kind="ExternalInput").ap()
    bqkvT = nc.dram_tensor("bqkvT", [128, FTOT // 128], f32,
                           kind="ExternalInput").ap()
    woutT = nc.dram_tensor("woutT", [128, 4, E], bf16, kind="ExternalInput").ap()
    tabsD = nc.dram_tensor("tabs", [128, 4, TOK], bf16, kind="ExternalInput").ap()
    masksD = nc.dram_tensor("masks", [128, 3, 128], bf16, kind="ExternalInput").ap()
    esinkD = nc.dram_tensor("esink", [1, QH * 128], bf16, kind="ExternalInput").ap()
    permD = nc.dram_tensor("perm", [128, 128], bf16, kind="ExternalInput").ap()
    outT = nc.dram_tensor("outT", [128, NE, NQ], bf16, kind="ExternalOutput").ap()
    if DEBUG:
        dbgKR = nc.dram_tensor("dbgKR", [128, TOK], bf16, kind="ExternalOutput").ap()
        dbgQG = nc.dram_tensor("dbgQG", [128, 4, NQ], bf16, kind="ExternalOutput").ap()
        dbgVt = nc.dram_tensor("dbgVt", [128, 9, KG, 65], bf16, kind="ExternalOutput").ap()
        dbgCT = nc.dram_tensor("dbgCT", [128, 4, 2, 512], bf16, kind="ExternalOutput").ap()
        dbgP = nc.dram_tensor("dbgP", [128, 2, 512], bf16, kind="ExternalOutput").ap()
        dbgC = nc.dram_tensor("dbgC", [128, 512], f32, kind="ExternalOutput").ap()

    with tile.TileContext(nc) as tc:
        from contextlib import ExitStack
        es = ExitStack()
        with es:
            persist = es.enter_context(tc.tile_pool(name="persist", bufs=1))

            # ---- persistent SBUF tiles ----
            x_sb = persist.tile([128, NE, TOK], bf16)
            W_sb = persist.tile([128, 3, NE, 256], bf16)
            Wo = persist.tile([128, 4, E], bf16)
            tabs = persist.tile([128, 4, TOK], bf16)
            b_sb = persist.tile([128, FTOT // 128], f32)
            masks = persist.tile([128, 3, 128], bf16)
            esink = persist.tile([1, QH * 128], bf16)
            perm = persist.tile([128, 128], bf16)
            Ksb = persist.tile([128, TOK], bf16)
            Vsb = persist.tile([128, TOK], bf16)
            KR = persist.tile([128, TOK], bf16)
            QG = persist.tile([128, 4, NQ], bf16)
            Vtok = persist.tile([128, 9, KG, 65], bf16)
            ctxT = persist.tile([128, 4, 2, 512], bf16)
            ident = persist.tile([128, 128], f32)
            identb = persist.tile([128, 128], bf16)
            Irep = persist.tile([128, 4, 128], bf16)
            unit65 = persist.tile([1, 65], bf16)
            ones64 = persist.tile([1, 64], bf16)

            # ---- input DMAs: W fg0 halves + x chunks on both HWDGE queues --
            nc.sync.dma_start(W_sb[:, 0, 0:8], wqkvT[:, 0, 0:8])
            nc.scalar.dma_start(W_sb[:, 0, 8:16], wqkvT[:, 0, 8:16])
            for c in range(8):
                eng = nc.sync if c % 2 == 0 else nc.scalar
                eng.dma_start(x_sb[:, 2 * c:2 * c + 2, :],
                              xT[:, 2 * c:2 * c + 2, :])
            nc.sync.dma_start(tabs, tabsD)
            nc.sync.dma_start(perm, permD)
            nc.sync.dma_start(b_sb, bqkvT)
            nc.sync.dma_start(masks, masksD)
            nc.sync.dma_start(esink, esinkD)
            nc.sync.dma_start(W_sb[:, 1], wqkvT[:, 1])
            nc.sync.dma_start(W_sb[:, 2], wqkvT[:, 2])
            nc.sync.dma_start(Wo, woutT)

            # ---- on-device constants ----
            make_identity(nc, ident)
            nc.vector.tensor_copy(identb, ident)
            for j in range(4):
                nc.vector.tensor_copy(Irep[:, j, :], identb)
            nc.gpsimd.memset(unit65, 0.0)
            nc.gpsimd.memset(unit65[0:1, 64:65], 1.0)
            nc.gpsimd.memset(ones64, 1.0)
            nc.gpsimd.memset(Vtok[:, :, :, 64:65], 1.0)

            # ================= phase 1: K/V projection (e-outer) ==========
            es1 = ExitStack()
            psKV = es1.enter_context(
                tc.tile_pool(name="psKV", bufs=1, space="PSUM"))
            kvt = [[psKV.tile([128, 384], f32, tag=f"kv{f}{t}",
                              name=f"kv{f}{t}") for t in range(3)]
                   for f in range(2)]
            for e in range(NE):
                for f in range(2):
                    for t in range(3):
                        nc.tensor.matmul(
                            kvt[f][t],
                            W_sb[:, 0, e, 128 * f:128 * (f + 1)],
                            x_sb[:, e, 384 * t:384 * (t + 1)],
                            start=(e == 0), stop=(e == NE - 1))
            for t in range(3):
                nc.scalar.activation(out=Ksb[:, 384 * t:384 * (t + 1)],
                                     in_=kvt[0][t], func=Ident,
                                     bias=b_sb[:, 0:1])
            for t in range(3):
                nc.scalar.activation(out=Vsb[:, 384 * t:384 * (t + 1)],
                                     in_=kvt[1][t], func=Ident,
                                     bias=b_sb[:, 1:2])
            es1.close()

            # ====== phase 2: Q projection (no halo) + RoPE + V transpose ==
            es2 = ExitStack()
            psQ = es2.enter_context(
                tc.tile_pool(name="psQ", bufs=2, space="PSUM"))
            psR = es2.enter_context(
                tc.tile_pool(name="psR", bufs=2, space="PSUM"))
            psT = es2.enter_context(
                tc.tile_pool(name="psT", bufs=2, space="PSUM"))
            qsb_pool = es2.enter_context(tc.tile_pool(name="qsb", bufs=2))
            rsc = es2.enter_context(tc.tile_pool(name="rsc", bufs=3))

            def q_block_mm(b):
                pq = [psQ.tile([128, 512], f32, tag=f"q{t}", name=f"q{t}")
                      for t in range(2)]
                for t in range(2):
                    for e in range(NE):
                        nc.tensor.matmul(
                            pq[t],
                            W_sb[:, 1 + b // 2, e, 128 * (b % 2):128 * (b % 2 + 1)],
                            x_sb[:, e, 128 + 512 * t:128 + 512 * (t + 1)],
                            start=(e == 0), stop=(e == NE - 1))
                return pq

            def rope(src, nch, chw, toff, ci, si, dst_fn):
                """dst = src*cos + (perm@src)*sin over nch chunks of chw."""
                for ch in range(nch):
                    cs = slice(chw * ch, chw * (ch + 1))
                    ts = slice(toff + chw * ch, toff + chw * (ch + 1))
                    rot = psR.tile([128, 512], f32, tag="rot", name="rot")
                    nc.tensor.matmul(rot[:, 0:chw], perm, src[:, cs],
                                     start=True, stop=True)
                    m1 = rsc.tile([128, 512], bf16, tag="m1", name="m1")
                    nc.vector.tensor_mul(m1[:, 0:chw], src[:, cs],
                                         tabs[:, ci, ts])
                    m2 = rsc.tile([128, 512], bf16, tag="m2", name="m2")
                    nc.vector.tensor_mul(m2[:, 0:chw], rot[:, 0:chw],
                                         tabs[:, si, ts])
                    nc.gpsimd.tensor_add(dst_fn(cs), m1[:, 0:chw],
                                         m2[:, 0:chw])

            def q_drain_rope(b, pq):
                qsb = qsb_pool.tile([128, NQ], bf16, tag="qsb", name="qsb")
                for t in range(2):
                    nc.scalar.activation(
                        out=qsb[:, 512 * t:512 * (t + 1)], in_=pq[t],
                        func=Ident, bias=b_sb[:, 2 + b:3 + b])
                rope(qsb, 2, 512, 128, 2, 3, lambda cs: QG[:, b, cs])

            def v_transpose(kbs):
                for kb in kbs:
                    for g in range(KG):
                        pt = psT.tile([128, 64], f32, tag="vt", name="vt")
                        ptb = pt.bitcast(bf16)
                        nc.tensor.transpose(
                            ptb[:, 0:64],
                            Vsb[64 * g:64 * (g + 1), 128 * kb:128 * (kb + 1)],
                            identb[64 * g:64 * (g + 1), 64 * g:64 * (g + 1)])
                        nc.vector.tensor_copy(Vtok[:, kb, g, 0:64],
                                              ptb[:, 0:64])

            pq0 = q_block_mm(0)
            rope(Ksb, 3, 384, 0, 0, 1, lambda cs: KR[:, cs])
            pq1 = q_block_mm(1)
            q_drain_rope(0, pq0)
            v_transpose(range(0, 5))
            pq2 = q_block_mm(2)
            q_drain_rope(1, pq1)
            v_transpose(range(5, 9))
            pq3 = q_block_mm(3)
            q_drain_rope(2, pq2)
            q_drain_rope(3, pq3)
            es2.close()

            # ============ phase 3: attention + out-projection =============
            psS = es.enter_context(
                tc.tile_pool(name="psS", bufs=2, space="PSUM"))
            psC = es.enter_context(
                tc.tile_pool(name="psC", bufs=2, space="PSUM"))
            psO = es.enter_context(
                tc.tile_pool(name="psO", bufs=2, space="PSUM"))
            pp = es.enter_context(tc.tile_pool(name="pp", bufs=3))
            pcx = es.enter_context(tc.tile_pool(name="pcx", bufs=3))
            prr = es.enter_context(tc.tile_pool(name="prr", bufs=2))
            prv = es.enter_context(tc.tile_pool(name="prv", bufs=2))
            po = es.enter_context(tc.tile_pool(name="po", bufs=2))

            if DEBUG:
                nc.sync.dma_start(dbgKR, KR)
                nc.sync.dma_start(dbgQG, QG)
                nc.sync.dma_start(dbgVt, Vtok)

            def attn_scores(qb, g):
                """Scores + mask + exp for 4 heads of kv-group g, block qb."""
                ps = psS.tile([128, 2, 512], f32, tag="ps", name="ps")
                qsl = QG[64 * g:64 * (g + 1), :, 128 * qb:128 * (qb + 1)]
                for kb in range(2):
                    mv = 0 if (kb == 0 and qb == 0) else (1 if kb == 0 else 2)
                    nc.tensor.matmul(ps[:, kb, :], masks[:, mv, :], Irep,
                                     start=True, stop=False)
                    nc.tensor.matmul(
                        ps[:, kb, :],
                        KR[64 * g:64 * (g + 1),
                           128 * (qb + kb):128 * (qb + kb + 1)],
                        qsl, start=False, stop=True)
                p = pp.tile([128, 2, 512], bf16, tag="p", name="p")
                nc.scalar.activation(out=p, in_=ps, func=Exp)
                if DEBUG and qb == 1 and g == 0:
                    nc.sync.dma_start(dbgP, p)
                return p

            def attn_ctx(qb, g, p):
                """ctx + denom + normalize into ctxT."""
                pc = psC.tile([128, 512], f32, tag="pc", name="pc")
                nc.tensor.matmul(pc[0:65, :], unit65,
                                 esink[0:1, 512 * g:512 * (g + 1)],
                                 start=True, stop=False)
                nc.tensor.matmul(pc[0:65, :], Vtok[:, qb, g, :], p[:, 0, :],
                                 start=False, stop=False)
                nc.tensor.matmul(pc[0:65, :], Vtok[:, qb + 1, g, :],
                                 p[:, 1, :], start=False, stop=True)
                if DEBUG and qb == 1 and g == 0:
                    cpy = pcx.tile([128, 512], f32, tag="dbgc", name="dbgc")
                    nc.scalar.activation(out=cpy, in_=pc, func=Ident)
                    nc.sync.dma_start(dbgC, cpy)
                rinv = prv.tile([1, 512], bf16, tag="rv", name="rv")
                with nc.allow_low_precision(reason="bf16 1/denom is plenty"):
                    nc.vector.reciprocal(rinv, pc[64:65, :])
                rr = prr.tile([64, 512], bf16, tag="rr", name="rr")
                nc.gpsimd.partition_broadcast(rr, rinv)
                cx = pcx.tile([64, 512], bf16, tag="cx", name="cx")
                nc.scalar.activation(out=cx, in_=pc[0:64, :], func=Ident)
                th, qq = qb // 4, qb % 4
                for hh in range(4):
                    pair, half = 2 * g + hh // 2, hh % 2
                    eng = nc.vector if half == 0 else nc.gpsimd
                    eng.tensor_mul(
                        ctxT[64 * half:64 * (half + 1), pair, th,
                             128 * qq:128 * (qq + 1)],
                        cx[:, 128 * hh:128 * (hh + 1)],
                        rr[:, 128 * hh:128 * (hh + 1)])

            oq = [nc.sync, nc.scalar]

            def outproj_eq(th, eq):
                o_sb = po.tile([128, 4, 512], bf16, tag="o", name="o")
                for el in range(4):
                    e = 4 * eq + el
                    pso = psO.tile([128, 512], f32, tag="po", name="po")
                    for h4 in range(4):
                        nc.tensor.matmul(pso, Wo[:, h4, 128 * e:128 * (e + 1)],
                                         ctxT[:, h4, th, :],
                                         start=(h4 == 0), stop=(h4 == 3))
                    if el % 2 == 0:
                        nc.scalar.activation(out=o_sb[:, el, :], in_=pso,
                                             func=Ident)
                    else:
                        nc.vector.tensor_copy(o_sb[:, el, :], pso)
                oq[(th + eq) % 2].dma_start(
                    outT[:, 4 * eq:4 * eq + 4, 512 * th:512 * (th + 1)], o_sb)

            # software-pipelined emission: scores(k) ahead of ctx(k-1);
            # out-projection th0 woven between th1 attention groups.
            groups = [(qb, g) for qb in range(8) for g in range(2)]
            pend = None
            for i, grp in enumerate(groups):
                p = attn_scores(*grp)
                if pend is not None:
                    attn_ctx(*pend[0], pend[1])
                    j = i - 1
                    if j in (7, 9, 11, 13):
                        outproj_eq(0, (j - 7) // 2)
                pend = (grp, p)
            attn_ctx(*pend[0], pend[1])
            for eq in range(4):
                outproj_eq(1, eq)
            if DEBUG:
                nc.scalar.dma_start(dbgCT, ctxT)

    nc.compile()
    return nc


# ---------------- host-side prep ----------------

def _rope_tables(position_ids, gstart):
    pos = np.zeros(TOK, dtype=np.float32)
    idx = gstart + np.arange(TOK)
    valid = (idx >= 0) & (idx < S)
    pos[valid] = position_ids[0, idx[valid]].astype(np.float32)
    freqs = (1.0 / ROPE_BASE ** (np.arange(0, D, 2, dtype=np.float32) / D)).astype(np.float32)
    wave_len = 2.0 * np.pi / freqs
    low = ORIG_CTX / BETA_FAST
    high = ORIG_CTX / BETA_SLOW
    t = np.clip((wave_len - low) / (high - low), 0.0, 1.0)
    eff = freqs * (1.0 - t) + (freqs / YARN_SCALE) * t
    conc = 0.1 * np.log(np.float32(YARN_SCALE)) + 1.0
    ang = pos[:, None] * eff[None, :] * conc
    sin = np.sin(ang).astype(np.float32).T    # [32, TOK]
    cos = np.cos(ang).astype(np.float32).T
    cosT = np.concatenate([cos, cos], axis=0)  # [64, TOK]
    sinS = np.concatenate([-sin, sin], axis=0)
    cos2 = np.concatenate([cosT, cosT], axis=0)  # [128, TOK]
    sinS2 = np.concatenate([sinS, sinS], axis=0)
    return np.ascontiguousarray(cos2), np.ascontiguousarray(sinS2)


def _build_masks(s):
    """Additive mask matrices, transposed for the PE mask-add:
    M_store[q, kv] = MASKVAL where kv is invalid for q.
    var0: block A for qb==0; var1: block A std; var2: block B."""
    q = np.arange(128)[:, None]
    kv = np.arange(128)[None, :]
    m_a = np.where(kv <= q, MASKVAL, 0.0).astype(np.float32)   # A: valid kv>q
    m_b = np.where(kv > q, MASKVAL, 0.0).astype(np.float32)    # B: valid kv<=q
    m_a0 = np.full((128, 128), MASKVAL, dtype=np.float32) if s == 0 else m_a
    return np.stack([m_a0, m_a, m_b], axis=1)                  # [128, 3, 128]


def _perm_matrix():
    """lhsT for rotate-half: out[p] = src[p xor 32] within each 64-half."""
    P = np.zeros((128, 128), dtype=np.float32)
    for m in range(128):
        half = (m // 64) * 64
        pi = half + ((m - half) + 32) % 64
        P[pi, m] = 1.0
    return P


def _prep_core(c, position_ids, Wqkv, bqkv, Wout, sinks, xT_full):
    s, h = c // 4, c % 4
    gstart = 1024 * s - 128
    xTc = np.zeros((E, TOK), dtype=np.float32)
    lo = max(0, gstart)
    xTc[:, lo - gstart:TOK] = xT_full[:, lo:gstart + TOK]
    # feature rows: K (2 groups), V (2 groups), Q blocks b = heads (b, 4+b)
    krows = np.arange(H * D + 128 * h, H * D + 128 * h + 128)
    vrows = np.arange((H + G) * D + 128 * h, (H + G) * D + 128 * h + 128)
    qrows = []
    for b in range(4):
        for l in (b, 4 + b):
            g_head = 8 * h + l
            qrows.append(np.arange(64 * g_head, 64 * g_head + 64))
    qrows = np.concatenate(qrows)
    rows = np.concatenate([krows, vrows, qrows])
    WqkvTc = np.ascontiguousarray(Wqkv[rows].T)
    bq = bqkv[rows].reshape(FTOT // 128, 128).T
    WoutTc = np.ascontiguousarray(Wout[:, 512 * h:512 * h + 512].T)
    cos2, sinS2 = _rope_tables(position_ids, gstart)
    masks = _build_masks(s)
    # esink: [1, 8*128], local head l = 4g + hh at block l
    es_l = np.exp(sinks[0, 8 * h:8 * h + 8, 0, 0]).astype(np.float32)
    esink = np.repeat(es_l, 128)[None, :]
    import ml_dtypes
    bf = ml_dtypes.bfloat16
    xP = xTc.reshape(NE, 128, TOK).transpose(1, 0, 2)
    wP = (WqkvTc.reshape(NE, 128, FTOT).transpose(1, 0, 2)
          .reshape(128, NE, 3, 256).transpose(0, 2, 1, 3))
    woP = WoutTc.reshape(4, 128, E).transpose(1, 0, 2)
    tabs = np.stack([cos2, sinS2, SCALE * cos2, SCALE * sinS2], axis=1)
    return {
        "xT": np.ascontiguousarray(xP.astype(bf)),
        "wqkvT": np.ascontiguousarray(wP.astype(bf)),
        "bqkvT": np.ascontiguousarray(bq.astype(np.float32)),
        "woutT": np.ascontiguousarray(woP.astype(bf)),
        "tabs": np.ascontiguousarray(tabs.astype(bf)),
        "masks": np.ascontiguousarray(masks.astype(bf)),
        "esink": np.ascontiguousarray(esink.astype(bf)),
        "perm": np.ascontiguousarray(_perm_matrix().astype(bf)),
    }


def _prep_all(inputs):
    x = np.asarray(inputs["x"], dtype=np.float32)
    position_ids = np.asarray(inputs["position_ids"])
    Wqkv = np.asarray(inputs["Wqkv"], dtype=np.float32)
    bqkv = np.asarray(inputs["bqkv"], dtype=np.float32)
    Wout = np.asarray(inputs["Wout"], dtype=np.float32)
    sinks = np.asarray(inputs["sinks"], dtype=np.float32)
    xT_full = np.ascontiguousarray(x[0].T)
    return [
        _prep_core(c, position_ids, Wqkv, bqkv, Wout, sinks, xT_full)
        for c in range(NCORES)
    ]


def kernel(x, position_ids, attn_mask, Wqkv, bqkv, Wout, bout, sinks):
    global _compiled
    from concourse.bass_utils import run_bass_kernel_spmd

    bout = np.asarray(bout, dtype=np.float32)

    if _compiled is None:
        _compiled = _build_bass()
    nc = _compiled

    in_maps = _prep_all({
        "x": x, "position_ids": position_ids,
        "Wqkv": Wqkv, "bqkv": bqkv, "Wout": Wout, "sinks": sinks,
    })
    res = run_bass_kernel_spmd(nc, in_maps, list(range(NCORES)))

    out = np.empty((S, E), dtype=np.float32)
    for s in range(2):
        acc = res.results[4 * s]["outT"].astype(np.float32)
        for h in range(1, 4):
            acc = acc + res.results[4 * s + h]["outT"].astype(np.float32)
        out[1024 * s:1024 * (s + 1)] = acc.transpose(1, 0, 2).reshape(E, NQ).T
    out += bout[None, :]
    return out[None]


# revision 79
# speedup vs baseline: 1.9549x; 1.9549x over previous
"""Trainium2 Bass kernel for nn_GroupQueryAttention_51616916963669.

GQA with YaRN RoPE, sliding-window (128) + causal mask, learned sink logit,
qkv/out projections. B=1, S=2048, E=2048, H=32, G=8, D=64.

Sharding over 8 NeuronCores: 2-way sequence (1024 queries each, with a
128-token KV halo) x 4-way heads (8 q-heads / 2 kv-groups each). Each core
computes a partial out-projection (over its 512 ctx dims); the host sums the
4 head-partials per sequence half and concatenates.

Design notes (transposed-scores attention):
- x streams in 8 chunked DMAs over both HWDGE queues; the K/V projection
  runs e-outer so the PE consumes chunks as they land (no big DMA wait).
- Q projection skips the 128-token halo (only K/V need it).
- Scores are computed TRANSPOSED ([kv, q]): psum = K^T Q per kv-block with
  the triangular band mask added via an extra matmul (lhsT=mask^T,
  rhs=replicated identity) into the same PSUM bank, so exp (ACT) reads a
  fully-masked tile and no DVE masking / row-sums / PE transposes exist.
- V is extended with a ones-column so the ctx matmul yields the softmax
  denominator for free in psum row 64; exp(sink) is preloaded into that
  row by a unit-row matmul. The denominator row is replicated across 64
  partitions by a bf16 ones-matmul, inverted with a 64-lane
  reciprocal_approx_fast, and the normalize runs as four GpSimd [64,128]
  multiplies writing bf16 ctxT directly (GpSimd may shift partitions for
  the odd heads). No gpsimd extended-library ops (library swaps cost ~8us).
- Emission is software-pipelined: scores(i) | ctx(i-1) | norm(i-2), with
  the out-projection emitted as 256-column pieces woven in as soon as
  each qb-pair's ctxT completes; outputs leave in 8 half-chunk DMAs
  alternating between the two queues.
"""
import numpy as np

# ---- problem constants (hardcoded per contract) ----
B, S, E = 1, 2048, 2048
H, G, D = 32, 8, 64
SW = 128
ROPE_BASE = 10000.0
ORIG_CTX = 4096.0
YARN_SCALE = 2.0
BETA_FAST, BETA_SLOW = 32.0, 1.0

# ---- sharding constants ----
NCORES = 8
TOK = 1152           # local kv tokens (9 blocks of 128)
NQ = 1024            # local query tokens (kv blocks 1..8)
QH = 8               # q heads per core
KG = 2               # kv groups per core
FTOT = QH * D + 2 * KG * D   # 768, feature order [K, V, Q0..Q3]
NE = E // 128        # 16 e-chunks
SCALE = 1.0 / (D ** 0.5)
MASKVAL = -30.0      # additive mask logit (exp(-30) ~ 1e-13)

_compiled = None
DEBUG = False


def _build_bass():
    import concourse.bacc as bacc
    import concourse.tile as tile
    import concourse.mybir as mybir
    from concourse.masks import make_identity

    f32 = mybir.dt.float32
    bf16 = mybir.dt.bfloat16
    Exp = mybir.ActivationFunctionType.Exp
    Ident = mybir.ActivationFunctionType.Identity

    nc = bacc.Bacc("TRN2", target_bir_lowering=False, debug=False,
                   num_devices=NCORES)

    xT = nc.dram_tensor("xT", [128, NE, TOK], bf16, kind="ExternalInput").ap()
    wqkvT = nc.dram_tensor("wqkvT", [128, 3, NE, 256], bf16,
                           kind="ExternalInput").ap()
    bqkvT = nc.dram_tensor("bqkvT", [128, FTOT // 128], f32,
                           kind="ExternalInput").ap()
    woutT = nc.dram_tensor("woutT", [128, 4, E], bf16, kind="ExternalInput").ap()
    tabsD = nc.dram_tensor("tabs", [128, 4, TOK], bf16, kind="ExternalInput").ap()
    masksD = nc.dram_tensor("masks", [128, 3, 128], bf16, kind="ExternalInput").ap()
    esinkD = nc.dram_tensor("esink", [1, QH * 128], bf16, kind="ExternalInput").ap()
    permD = nc.dram_tensor("perm", [128, 128], bf16, kind="ExternalInput").ap()
    outT = nc.dram_tensor("outT", [128, NE, NQ], bf16, kind="ExternalOutput").ap()
    if DEBUG:
        dbgKR = nc.dram_tensor("dbgKR", [128, TOK], bf16, kind="ExternalOutput").ap()
        dbgQG = nc.dram_tensor("dbgQG", [128, 4, NQ], bf16, kind="ExternalOutput").ap()
        dbgVt = nc.dram_tensor("dbgVt", [128, 9, KG, 65], bf16, kind="ExternalOutput").ap()
        dbgCT = nc.dram_tensor("dbgCT", [128, 4, 2, 512], bf16, kind="ExternalOutput").ap()
        dbgP = nc.dram_tensor("dbgP", [128, 2, 512], bf16, kind="ExternalOutput").ap()
        dbgC = nc.dram_tensor("dbgC", [128, 512], f32, kind="ExternalOutput").ap()

    with tile.TileContext(nc) as tc:
        from contextlib import ExitStack
        es = ExitStack()
        with es:
            persist = es.enter_context(tc.tile_pool(name="persist", bufs=1))

            # ---- persistent SBUF tiles ----
            x_sb = persist.tile([128, NE, TOK], bf16)
            W_sb = persist.tile([128, 3, NE, 256], bf16)
            Wo = persist.tile([128, 4, E], bf16)
            tabs = persist.tile([128, 4, TOK], bf16)
            b_sb = persist.tile([128, FTOT // 128], f32)
            masks = persist.tile([128, 3, 128], bf16)
            esink = persist.tile([1, QH * 128], bf16)
            perm = persist.tile([128, 128], bf16)
            Ksb = persist.tile([128, TOK], bf16)
            Vsb = persist.tile([128, TOK], bf16)
            KR = persist.tile([128, TOK], bf16)
            QG = persist.tile([128, 4, NQ], bf16)
            Vtok = persist.tile([128, 9, KG, 65], bf16)
            ctxT = persist.tile([128, 4, 2, 512], bf16)
            ident = persist.tile([128, 128], f32)
            identb = persist.tile([128, 128], bf16)
            Irep = persist.tile([128, 4, 128], bf16)
            unit65 = persist.tile([1, 65], bf16)
            ones64 = persist.tile([1, 64], bf16)

            # ---- input DMAs: W fg0 halves + x chunks on both HWDGE queues --
            nc.scalar.dma_start(x_sb[:, 0:2, :], xT[:, 0:2, :])
            nc.sync.dma_start(W_sb[:, 0, 0:8], wqkvT[:, 0, 0:8])
            nc.scalar.dma_start(W_sb[:, 0, 8:16], wqkvT[:, 0, 8:16])
            for c in range(1, 8):
                eng = nc.sync if c % 2 == 1 else nc.scalar
                eng.dma_start(x_sb[:, 2 * c:2 * c + 2, :],
                              xT[:, 2 * c:2 * c + 2, :])
            nc.sync.dma_start(tabs, tabsD)
            nc.sync.dma_start(perm, permD)
            nc.sync.dma_start(b_sb, bqkvT)
            nc.sync.dma_start(masks, masksD)
            nc.sync.dma_start(esink, esinkD)
            nc.sync.dma_start(W_sb[:, 1], wqkvT[:, 1])
            nc.sync.dma_start(W_sb[:, 2], wqkvT[:, 2])
            nc.sync.dma_start(Wo, woutT)

            # ---- on-device constants ----
            make_identity(nc, ident)
            nc.vector.tensor_copy(identb, ident)
            for j in range(4):
                nc.vector.tensor_copy(Irep[:, j, :], identb)
            nc.gpsimd.memset(unit65, 0.0)
            nc.gpsimd.memset(unit65[0:1, 64:65], 1.0)
            nc.gpsimd.memset(ones64, 1.0)
            nc.gpsimd.memset(Vtok[:, :, :, 64:65], 1.0)

            # ================= phase 1: K/V projection (e-outer) ==========
            es1 = ExitStack()
            psKV = es1.enter_context(
                tc.tile_pool(name="psKV", bufs=1, space="PSUM"))
            kvt = [[psKV.tile([128, 384], f32, tag=f"kv{f}{t}",
                              name=f"kv{f}{t}") for t in range(3)]
                   for f in range(2)]
            for e in range(NE):
                for f in range(2):
                    for t in range(3):
                        nc.tensor.matmul(
                            kvt[f][t],
                            W_sb[:, 0, e, 128 * f:128 * (f + 1)],
                            x_sb[:, e, 384 * t:384 * (t + 1)],
                            start=(e == 0), stop=(e == NE - 1))
            for t in range(3):
                nc.scalar.activation(out=Ksb[:, 384 * t:384 * (t + 1)],
                                     in_=kvt[0][t], func=Ident,
                                     bias=b_sb[:, 0:1])
            for t in range(3):
                nc.scalar.activation(out=Vsb[:, 384 * t:384 * (t + 1)],
                                     in_=kvt[1][t], func=Ident,
                                     bias=b_sb[:, 1:2])
            es1.close()

            # ====== phase 2a: Q0/Q1 projection + RoPE K + V transpose =====
            qsb_pool = es.enter_context(tc.tile_pool(name="qsb", bufs=2))
            rsc = es.enter_context(tc.tile_pool(name="rsc", bufs=3))
            es2 = ExitStack()
            psQ = es2.enter_context(
                tc.tile_pool(name="psQ", bufs=2, space="PSUM"))
            psR = es2.enter_context(
                tc.tile_pool(name="psR", bufs=2, space="PSUM"))
            psT = es2.enter_context(
                tc.tile_pool(name="psT", bufs=2, space="PSUM"))

            def q_block_mm(b, pool, tag):
                pq = [pool.tile([128, 512], f32, tag=f"{tag}{t}",
                                name=f"{tag}{t}") for t in range(2)]
                for t in range(2):
                    for e in range(NE):
                        nc.tensor.matmul(
                            pq[t],
                            W_sb[:, 1 + b // 2, e, 128 * (b % 2):128 * (b % 2 + 1)],
                            x_sb[:, e, 128 + 512 * t:128 + 512 * (t + 1)],
                            start=(e == 0), stop=(e == NE - 1))
                return pq

            def rope(src, nch, chw, toff, ci, si, add_fn, pool, tag):
                """dst = src*cos + (perm@src)*sin over nch chunks of chw."""
                for ch in range(nch):
                    cs = slice(chw * ch, chw * (ch + 1))
                    ts = slice(toff + chw * ch, toff + chw * (ch + 1))
                    rot = pool.tile([128, 512], f32, tag=tag, name=tag)
                    nc.tensor.matmul(rot[:, 0:chw], perm, src[:, cs],
                                     start=True, stop=True)
                    m1 = rsc.tile([128, 512], bf16, tag="m1", name="m1")
                    nc.vector.tensor_mul(m1[:, 0:chw], src[:, cs],
                                         tabs[:, ci, ts])
                    m2 = rsc.tile([128, 512], bf16, tag="m2", name="m2")
                    nc.vector.tensor_mul(m2[:, 0:chw], rot[:, 0:chw],
                                         tabs[:, si, ts])
                    add_fn(cs, m1[:, 0:chw], m2[:, 0:chw])

            def k_add(cs, m1, m2):
                nc.gpsimd.tensor_add(KR[:, cs], m1, m2)

            def q_drain_rope(b, pq, pool, tag):
                qsb = qsb_pool.tile([128, NQ], bf16, tag="qsb", name="qsb")
                for t in range(2):
                    nc.scalar.activation(
                        out=qsb[:, 512 * t:512 * (t + 1)], in_=pq[t],
                        func=Ident, bias=b_sb[:, 2 + b:3 + b])
                rope(qsb, 2, 512, 128, 2, 3,
                     lambda cs, m1, m2: nc.gpsimd.tensor_add(QG[:, b, cs],
                                                             m1, m2),
                     pool, tag)

            def v_transpose(kbs):
                for kb in kbs:
                    for g in range(KG):
                        pt = psT.tile([128, 64], f32, tag="vt", name="vt")
                        ptb = pt.bitcast(bf16)
                        nc.tensor.transpose(
                            ptb[:, 0:64],
                            Vsb[64 * g:64 * (g + 1), 128 * kb:128 * (kb + 1)],
                            identb[64 * g:64 * (g + 1), 64 * g:64 * (g + 1)])
                        nc.vector.tensor_copy(Vtok[:, kb, g, 0:64],
                                              ptb[:, 0:64])

            pq0 = q_block_mm(0, psQ, "q")
            rope(Ksb, 3, 384, 0, 0, 1, k_add, psR, "rot")
            pq1 = q_block_mm(1, psQ, "q")
            q_drain_rope(0, pq0, psR, "rot")
            v_transpose(range(0, 5))
            pq2 = q_block_mm(2, psQ, "q")
            q_drain_rope(1, pq1, psR, "rot")
            v_transpose(range(5, 9))
            pq3 = q_block_mm(3, psQ, "q")
            q_drain_rope(2, pq2, psR, "rot")
            q_drain_rope(3, pq3, psR, "rot")
            es2.close()

            # ============ phase 3: attention + out-projection + Q2/Q3 =====
            psS = es.enter_context(
                tc.tile_pool(name="psS", bufs=1, space="PSUM"))
            psC = es.enter_context(
                tc.tile_pool(name="psC", bufs=2, space="PSUM"))
            psRr = es.enter_context(
                tc.tile_pool(name="psRr", bufs=2, space="PSUM"))
            flex = es.enter_context(
                tc.tile_pool(name="flex", bufs=2, space="PSUM"))
            pp = es.enter_context(tc.tile_pool(name="pp", bufs=5))
            pcx = es.enter_context(tc.tile_pool(name="pcx", bufs=6))
            prv = es.enter_context(tc.tile_pool(name="prv", bufs=6))
            po = es.enter_context(tc.tile_pool(name="po", bufs=2))

            if DEBUG:
                nc.sync.dma_start(dbgKR, KR)
                nc.sync.dma_start(dbgQG, QG)
                nc.sync.dma_start(dbgVt, Vtok)

            def attn_scores(qb, g):
                """Scores + mask + exp for 4 heads of kv-group g, block qb."""
                ps = psS.tile([128, 2, 512], f32, tag="ps", name="ps")
                qsl = QG[64 * g:64 * (g + 1), :, 128 * qb:128 * (qb + 1)]
                for kb in range(2):
                    mv = 0 if (kb == 0 and qb == 0) else (1 if kb == 0 else 2)
                    nc.tensor.matmul(ps[:, kb, :], masks[:, mv, :], Irep,
                                     start=True, stop=False)
                    nc.tensor.matmul(
                        ps[:, kb, :],
                        KR[64 * g:64 * (g + 1),
                           128 * (qb + kb):128 * (qb + kb + 1)],
                        qsl, start=False, stop=True)
                p = pp.tile([128, 2, 512], bf16, tag="p", name="p")
                nc.scalar.activation(out=p[:, 0, :], in_=ps[:, 0, :],
                                     func=Exp)
                nc.scalar.activation(out=p[:, 1, :], in_=ps[:, 1, :],
                                     func=Exp)
                if DEBUG and qb == 1 and g == 0:
                    nc.sync.dma_start(dbgP, p)
                return ps, p

            def attn_ctx(qb, g, ps, p):
                """ctx for all 4 heads (+denom row 64) into psC; drains."""
                pc = psC.tile([128, 512], f32, tag="pc", name="pc")
                nc.tensor.matmul(pc[0:65, :], unit65,
                                 esink[0:1, 512 * g:512 * (g + 1)],
                                 start=True, stop=False)
                nc.tensor.matmul(pc[0:65, :], Vtok[:, qb, g, :],
                                 p[:, 0, :], start=False, stop=False)
                nc.tensor.matmul(pc[0:65, :], Vtok[:, qb + 1, g, :],
                                 p[:, 1, :], start=False, stop=True)
                if DEBUG and qb == 1 and g == 0:
                    cpy = pcx.tile([128, 512], f32, tag="dbgc", name="dbgc")
                    nc.scalar.activation(out=cpy, in_=pc, func=Ident)
                    nc.sync.dma_start(dbgC, cpy)
                cx = pcx.tile([64, 4, 128], bf16, tag="cx", name="cx")
                nc.vector.tensor_copy(cx, pc[0:64, :])
                dsb = prv.tile([1, 512], bf16, tag="dn", name="dn")
                nc.vector.tensor_copy(dsb, pc[64:65, :])
                return cx, dsb

            def attn_norm(qb, g, cx, dsb):
                """Replicate denom across 64 partitions (PE, bf16), 64-lane
                reciprocal, then scale ctx into ctxT (GpSimd, which may
                shift partitions for the odd heads)."""
                rr = psRr.tile([64, 4, 128], f32, tag="rr", name="rr")
                th, qq = qb // 4, qb % 4
                for hh in range(4):
                    nc.tensor.matmul(rr[:, hh, :], ones64,
                                     dsb[0:1, 128 * hh:128 * (hh + 1)],
                                     start=True, stop=True)
                rds = prv.tile([64, 4, 128], f32, tag="rd", name="rd")
                nc.scalar.activation(out=rds, in_=rr, func=Ident)
                rinv = prv.tile([64, 4, 128], f32, tag="rv", name="rv")
                nc.vector.reciprocal_approx_fast(rinv, rds)
                for hh in range(4):
                    pair, half = 2 * g + hh // 2, hh % 2
                    nc.gpsimd.tensor_mul(
                        ctxT[64 * half:64 * (half + 1), pair, th,
                             128 * qq:128 * (qq + 1)],
                        cx[:, hh, :], rinv[:, hh, :])

            oq = [nc.sync, nc.scalar]

            # out-projection in 4 chunks of 256 q-cols (one per qb-pair),
            # each split into 8 pieces of 2 e-chunks for weaving.
            op_osb = {}

            def op_piece(ci, pi):
                th, pr = ci // 2, ci % 2
                if pi == 0:
                    op_osb[ci] = po.tile([128, NE, 256], bf16,
                                         tag=f"o{ci % 2}", name=f"o{ci % 2}")
                o_sb = op_osb[ci]
                csl = slice(256 * pr, 256 * (pr + 1))
                pso = flex.tile([128, 512], f32, tag="fx", name="fx")
                for el in range(2):
                    e = 2 * pi + el
                    for h4 in range(4):
                        nc.tensor.matmul(pso[:, 256 * el:256 * (el + 1)],
                                         Wo[:, h4, 128 * e:128 * (e + 1)],
                                         ctxT[:, h4, th, csl],
                                         start=(h4 == 0), stop=(h4 == 3))
                for el in range(2):
                    osl = o_sb[:, 2 * pi + el, :]
                    pss = pso[:, 256 * el:256 * (el + 1)]
                    if pi % 2 == 0:
                        nc.scalar.activation(out=osl, in_=pss, func=Ident)
                    else:
                        nc.vector.tensor_copy(osl, pss)
                if pi in (3, 7):
                    eh = slice(8 * (pi // 4), 8 * (pi // 4) + 8)
                    oq[(ci + pi // 4) % 2].dma_start(
                        outT[:, eh, 512 * th + 256 * pr:
                             512 * th + 256 * (pr + 1)], o_sb[:, eh, :])

            # software-pipelined emission: scores(i) | ctx(i-1) | norm(i-2);
            # out-projection pieces woven in as their qb-pair completes.
            groups = [(qb, g) for qb in range(8) for g in range(2)]
            sc_out = {}
            cx_out = {}
            op_queue = []

            def emit_fill(i):
                n = 2
                for _ in range(n):
                    if not op_queue:
                        return
                    op_piece(*op_queue.pop(0))

            def after_norm(grp):
                qb, g = grp
                if g == 1 and qb % 2 == 1:
                    ci = qb // 2
                    op_queue.extend((ci, pi) for pi in range(8))

            for i, grp in enumerate(groups):
                sc_out[i] = attn_scores(*grp)
                if i >= 1:
                    cx_out[i - 1] = attn_ctx(*groups[i - 1], *sc_out.pop(i - 1))
                if i >= 2:
                    attn_norm(*groups[i - 2], *cx_out.pop(i - 2))
                    after_norm(groups[i - 2])
                emit_fill(i)
            cx_out[15] = attn_ctx(*groups[15], *sc_out.pop(15))
            attn_norm(*groups[14], *cx_out.pop(14))
            after_norm(groups[14])
            emit_fill(15)
            attn_norm(*groups[15], *cx_out.pop(15))
            after_norm(groups[15])
            while op_queue:
                op_piece(*op_queue.pop(0))
            if DEBUG:
                nc.scalar.dma_start(dbgCT, ctxT)

    nc.compile()
    return nc


# ---------------- host-side prep ----------------

def _rope_tables(position_ids, gstart):
    pos = np.zeros(TOK, dtype=np.float32)
    idx = gstart + np.arange(TOK)
    valid = (idx >= 0) & (idx < S)
    pos[valid] = position_ids[0, idx[valid]].astype(np.float32)
    freqs = (1.0 / ROPE_BASE ** (np.arange(0, D, 2, dtype=np.float32) / D)).astype(np.float32)
    wave_len = 2.0 * np.pi / freqs
    low = ORIG_CTX / BETA_FAST
    high = ORIG_CTX / BETA_SLOW
    t = np.clip((wave_len - low) / (high - low), 0.0, 1.0)
    eff = freqs * (1.0 - t) + (freqs / YARN_SCALE) * t
    conc = 0.1 * np.log(np.float32(YARN_SCALE)) + 1.0
    ang = pos[:, None] * eff[None, :] * conc
    sin = np.sin(ang).astype(np.float32).T    # [32, TOK]
    cos = np.cos(ang).astype(np.float32).T
    cosT = np.concatenate([cos, cos], axis=0)  # [64, TOK]
    sinS = np.concatenate([-sin, sin], axis=0)
    cos2 = np.concatenate([cosT, cosT], axis=0)  # [128, TOK]
    sinS2 = np.concatenate([sinS, sinS], axis=0)
    return np.ascontiguousarray(cos2), np.ascontiguousarray(sinS2)


def _build_masks(s):
    """Additive mask matrices, transposed for the PE mask-add:
    M_store[q, kv] = MASKVAL where kv is invalid for q.
    var0: block A for qb==0; var1: block A std; var2: block B."""
    q = np.arange(128)[:, None]
    kv = np.arange(128)[None, :]
    m_a = np.where(kv <= q, MASKVAL, 0.0).astype(np.float32)   # A: valid kv>q
    m_b = np.where(kv > q, MASKVAL, 0.0).astype(np.float32)    # B: valid kv<=q
    m_a0 = np.full((128, 128), MASKVAL, dtype=np.float32) if s == 0 else m_a
    return np.stack([m_a0, m_a, m_b], axis=1)                  # [128, 3, 128]


def _perm_matrix():
    """lhsT for rotate-half: out[p] = src[p xor 32] within each 64-half."""
    P = np.zeros((128, 128), dtype=np.float32)
    for m in range(128):
        half = (m // 64) * 64
        pi = half + ((m - half) + 32) % 64
        P[pi, m] = 1.0
    return P


def _prep_core(c, position_ids, Wqkv, bqkv, Wout, sinks, xT_full):
    s, h = c // 4, c % 4
    gstart = 1024 * s - 128
    xTc = np.zeros((E, TOK), dtype=np.float32)
    lo = max(0, gstart)
    xTc[:, lo - gstart:TOK] = xT_full[:, lo:gstart + TOK]
    # feature rows: K (2 groups), V (2 groups), Q blocks b = heads (b, 4+b)
    krows = np.arange(H * D + 128 * h, H * D + 128 * h + 128)
    vrows = np.arange((H + G) * D + 128 * h, (H + G) * D + 128 * h + 128)
    qrows = []
    for b in range(4):
        for l in (b, 4 + b):
            g_head = 8 * h + l
            qrows.append(np.arange(64 * g_head, 64 * g_head + 64))
    qrows = np.concatenate(qrows)
    rows = np.concatenate([krows, vrows, qrows])
    WqkvTc = np.ascontiguousarray(Wqkv[rows].T)
    bq = bqkv[rows].reshape(FTOT // 128, 128).T
    WoutTc = np.ascontiguousarray(Wout[:, 512 * h:512 * h + 512].T)
    cos2, sinS2 = _rope_tables(position_ids, gstart)
    masks = _build_masks(s)
    # esink: [1, 8*128], score-column head order [4g, 4g+2, 4g+1, 4g+3]
    es_l = np.exp(sinks[0, 8 * h:8 * h + 8, 0, 0]).astype(np.float32)
    esink = np.repeat(es_l, 128)[None, :]
    import ml_dtypes
    bf = ml_dtypes.bfloat16
    xP = xTc.reshape(NE, 128, TOK).transpose(1, 0, 2)
    wP = (WqkvTc.reshape(NE, 128, FTOT).transpose(1, 0, 2)
          .reshape(128, NE, 3, 256).transpose(0, 2, 1, 3))
    woP = WoutTc.reshape(4, 128, E).transpose(1, 0, 2)
    tabs = np.stack([cos2, sinS2, SCALE * cos2, SCALE * sinS2], axis=1)
    return {
        "xT": np.ascontiguousarray(xP.astype(bf)),
        "wqkvT": np.ascontiguousarray(wP.astype(bf)),
        "bqkvT": np.ascontiguousarray(bq.astype(np.float32)),
        "woutT": np.ascontiguousarray(woP.astype(bf)),
        "tabs": np.ascontiguousarray(tabs.astype(bf)),
        "masks": np.ascontiguousarray(masks.astype(bf)),
        "esink": np.ascontiguousarray(esink.astype(bf)),
        "perm": np.ascontiguousarray(_perm_matrix().astype(bf)),
    }


def _prep_all(inputs):
    x = np.asarray(inputs["x"], dtype=np.float32)
    position_ids = np.asarray(inputs["position_ids"])
    Wqkv = np.asarray(inputs["Wqkv"], dtype=np.float32)
    bqkv = np.asarray(inputs["bqkv"], dtype=np.float32)
    Wout = np.asarray(inputs["Wout"], dtype=np.float32)
    sinks = np.asarray(inputs["sinks"], dtype=np.float32)
    xT_full = np.ascontiguousarray(x[0].T)
    return [
        _prep_core(c, position_ids, Wqkv, bqkv, Wout, sinks, xT_full)
        for c in range(NCORES)
    ]


def kernel(x, position_ids, attn_mask, Wqkv, bqkv, Wout, bout, sinks):
    global _compiled
    from concourse.bass_utils import run_bass_kernel_spmd

    bout = np.asarray(bout, dtype=np.float32)

    if _compiled is None:
        _compiled = _build_bass()
    nc = _compiled

    in_maps = _prep_all({
        "x": x, "position_ids": position_ids,
        "Wqkv": Wqkv, "bqkv": bqkv, "Wout": Wout, "sinks": sinks,
    })
    res = run_bass_kernel_spmd(nc, in_maps, list(range(NCORES)))

    out = np.empty((S, E), dtype=np.float32)
    for s in range(2):
        acc = res.results[4 * s]["outT"].astype(np.float32)
        for h in range(1, 4):
            acc = acc + res.results[4 * s + h]["outT"].astype(np.float32)
        out[1024 * s:1024 * (s + 1)] = acc.transpose(1, 0, 2).reshape(E, NQ).T
    out += bout[None, :]
    return out[None]


# revision 80
# speedup vs baseline: 1.9801x; 1.0129x over previous
"""Trainium2 Bass kernel for nn_GroupQueryAttention_51616916963669.

GQA with YaRN RoPE, sliding-window (128) + causal mask, learned sink logit,
qkv/out projections. B=1, S=2048, E=2048, H=32, G=8, D=64.

Sharding over 8 NeuronCores: 2-way sequence (1024 queries each, with a
128-token KV halo) x 4-way heads (8 q-heads / 2 kv-groups each). Each core
computes a partial out-projection (over its 512 ctx dims); the host sums the
4 head-partials per sequence half and concatenates.

Design notes (transposed-scores attention):
- x streams in 8 chunked DMAs over both HWDGE queues; the K/V projection
  runs e-outer so the PE consumes chunks as they land (no big DMA wait).
- Q projection skips the 128-token halo (only K/V need it).
- Scores are computed TRANSPOSED ([kv, q]): psum = K^T Q per kv-block with
  the triangular band mask added via an extra matmul (lhsT=mask^T,
  rhs=replicated identity) into the same PSUM bank, so exp (ACT) reads a
  fully-masked tile and no DVE masking / row-sums / PE transposes exist.
- V is extended with a ones-column so the ctx matmul yields the softmax
  denominator for free in psum row 64; exp(sink) is preloaded into that
  row by a unit-row matmul. The denominator row is replicated across 64
  partitions by a bf16 ones-matmul, inverted with a 64-lane
  reciprocal_approx_fast, and the normalize runs as four GpSimd [64,128]
  multiplies writing bf16 ctxT directly (GpSimd may shift partitions for
  the odd heads). No gpsimd extended-library ops (library swaps cost ~8us).
- Emission is software-pipelined: scores(i) | ctx(i-1) | norm(i-2), with
  the out-projection emitted as 256-column pieces woven in as soon as
  each qb-pair's ctxT completes; outputs leave in 8 half-chunk DMAs
  alternating between the two queues.
"""
import numpy as np

# ---- problem constants (hardcoded per contract) ----
B, S, E = 1, 2048, 2048
H, G, D = 32, 8, 64
SW = 128
ROPE_BASE = 10000.0
ORIG_CTX = 4096.0
YARN_SCALE = 2.0
BETA_FAST, BETA_SLOW = 32.0, 1.0

# ---- sharding constants ----
NCORES = 8
TOK = 1152           # local kv tokens (9 blocks of 128)
NQ = 1024            # local query tokens (kv blocks 1..8)
QH = 8               # q heads per core
KG = 2               # kv groups per core
FTOT = QH * D + 2 * KG * D   # 768, feature order [K, V, Q0..Q3]
NE = E // 128        # 16 e-chunks
SCALE = 1.0 / (D ** 0.5)
MASKVAL = -30.0      # additive mask logit (exp(-30) ~ 1e-13)

_compiled = None
DEBUG = False


def _build_bass():
    import concourse.bacc as bacc
    import concourse.tile as tile
    import concourse.mybir as mybir
    from concourse.masks import make_identity

    f32 = mybir.dt.float32
    bf16 = mybir.dt.bfloat16
    Exp = mybir.ActivationFunctionType.Exp
    Ident = mybir.ActivationFunctionType.Identity

    nc = bacc.Bacc("TRN2", target_bir_lowering=False, debug=False,
                   num_devices=NCORES)

    xT = nc.dram_tensor("xT", [128, NE, TOK], bf16, kind="ExternalInput").ap()
    wqkvT = nc.dram_tensor("wqkvT", [128, 3, NE, 256], bf16,
                           kind="ExternalInput").ap()
    bqkvT = nc.dram_tensor("bqkvT", [128, FTOT // 128], f32,
                           kind="ExternalInput").ap()
    woutT = nc.dram_tensor("woutT", [128, 4, E], bf16, kind="ExternalInput").ap()
    tabsD = nc.dram_tensor("tabs", [128, 4, TOK], bf16, kind="ExternalInput").ap()
    masksD = nc.dram_tensor("masks", [128, 3, 128], bf16, kind="ExternalInput").ap()
    esinkD = nc.dram_tensor("esink", [1, QH * 128], bf16, kind="ExternalInput").ap()
    permD = nc.dram_tensor("perm", [128, 128], bf16, kind="ExternalInput").ap()
    outT = nc.dram_tensor("outT", [128, NE, NQ], bf16, kind="ExternalOutput").ap()
    if DEBUG:
        dbgKR = nc.dram_tensor("dbgKR", [128, TOK], bf16, kind="ExternalOutput").ap()
        dbgQG = nc.dram_tensor("dbgQG", [128, 4, NQ], bf16, kind="ExternalOutput").ap()
        dbgVt = nc.dram_tensor("dbgVt", [128, 9, KG, 65], bf16, kind="ExternalOutput").ap()
        dbgCT = nc.dram_tensor("dbgCT", [128, 4, 2, 512], bf16, kind="ExternalOutput").ap()
        dbgP = nc.dram_tensor("dbgP", [128, 2, 512], bf16, kind="ExternalOutput").ap()
        dbgC = nc.dram_tensor("dbgC", [128, 512], f32, kind="ExternalOutput").ap()

    with tile.TileContext(nc) as tc:
        from contextlib import ExitStack
        es = ExitStack()
        with es:
            persist = es.enter_context(tc.tile_pool(name="persist", bufs=1))

            # ---- persistent SBUF tiles ----
            x_sb = persist.tile([128, NE, TOK], bf16)
            W_sb = persist.tile([128, 3, NE, 256], bf16)
            Wo = persist.tile([128, 4, E], bf16)
            tabs = persist.tile([128, 4, TOK], bf16)
            b_sb = persist.tile([128, FTOT // 128], f32)
            masks = persist.tile([128, 3, 128], bf16)
            esink = persist.tile([1, QH * 128], bf16)
            perm = persist.tile([128, 128], bf16)
            Ksb = persist.tile([128, TOK], bf16)
            Vsb = persist.tile([128, TOK], bf16)
            KR = persist.tile([128, TOK], bf16)
            QG = persist.tile([128, 4, NQ], bf16)
            Vtok = persist.tile([128, 9, KG, 65], bf16)
            ctxT = persist.tile([128, 4, 2, 512], bf16)
            ident = persist.tile([128, 128], f32)
            identb = persist.tile([128, 128], bf16)
            Irep = persist.tile([128, 4, 128], bf16)
            unit65 = persist.tile([1, 65], bf16)
            ones64 = persist.tile([1, 64], bf16)

            # ---- input DMAs: W fg0 halves + x chunks on both HWDGE queues --
            nc.scalar.dma_start(x_sb[:, 0:1, :], xT[:, 0:1, :])
            nc.sync.dma_start(W_sb[:, 0, 0:4], wqkvT[:, 0, 0:4])
            nc.scalar.dma_start(x_sb[:, 1:2, :], xT[:, 1:2, :])
            nc.sync.dma_start(W_sb[:, 0, 4:10], wqkvT[:, 0, 4:10])
            nc.scalar.dma_start(W_sb[:, 0, 10:16], wqkvT[:, 0, 10:16])
            for c in range(1, 8):
                eng = nc.sync if c % 2 == 1 else nc.scalar
                eng.dma_start(x_sb[:, 2 * c:2 * c + 2, :],
                              xT[:, 2 * c:2 * c + 2, :])
            nc.sync.dma_start(tabs, tabsD)
            nc.sync.dma_start(perm, permD)
            nc.sync.dma_start(b_sb, bqkvT)
            nc.sync.dma_start(masks, masksD)
            nc.sync.dma_start(esink, esinkD)
            nc.sync.dma_start(W_sb[:, 1], wqkvT[:, 1])
            nc.sync.dma_start(W_sb[:, 2], wqkvT[:, 2])
            nc.sync.dma_start(Wo, woutT)

            # ---- on-device constants ----
            make_identity(nc, ident)
            nc.vector.tensor_copy(identb, ident)
            for j in range(4):
                nc.vector.tensor_copy(Irep[:, j, :], identb)
            nc.gpsimd.memset(unit65, 0.0)
            nc.gpsimd.memset(unit65[0:1, 64:65], 1.0)
            nc.gpsimd.memset(ones64, 1.0)
            nc.gpsimd.memset(Vtok[:, :, :, 64:65], 1.0)

            # ================= phase 1: K/V projection (e-outer) ==========
            es1 = ExitStack()
            psKV = es1.enter_context(
                tc.tile_pool(name="psKV", bufs=1, space="PSUM"))
            kvt = [[psKV.tile([128, 384], f32, tag=f"kv{f}{t}",
                              name=f"kv{f}{t}") for t in range(3)]
                   for f in range(2)]
            for e in range(NE):
                for f in range(2):
                    for t in range(3):
                        nc.tensor.matmul(
                            kvt[f][t],
                            W_sb[:, 0, e, 128 * f:128 * (f + 1)],
                            x_sb[:, e, 384 * t:384 * (t + 1)],
                            start=(e == 0), stop=(e == NE - 1))
            for t in range(3):
                nc.scalar.activation(out=Ksb[:, 384 * t:384 * (t + 1)],
                                     in_=kvt[0][t], func=Ident,
                                     bias=b_sb[:, 0:1])
            for t in range(3):
                nc.scalar.activation(out=Vsb[:, 384 * t:384 * (t + 1)],
                                     in_=kvt[1][t], func=Ident,
                                     bias=b_sb[:, 1:2])
            es1.close()

            # ====== phase 2a: Q0/Q1 projection + RoPE K + V transpose =====
            qsb_pool = es.enter_context(tc.tile_pool(name="qsb", bufs=2))
            rsc = es.enter_context(tc.tile_pool(name="rsc", bufs=3))
            es2 = ExitStack()
            psQ = es2.enter_context(
                tc.tile_pool(name="psQ", bufs=2, space="PSUM"))
            psR = es2.enter_context(
                tc.tile_pool(name="psR", bufs=2, space="PSUM"))
            psT = es2.enter_context(
                tc.tile_pool(name="psT", bufs=2, space="PSUM"))

            def q_block_mm(b, pool, tag):
                pq = [pool.tile([128, 512], f32, tag=f"{tag}{t}",
                                name=f"{tag}{t}") for t in range(2)]
                for t in range(2):
                    for e in range(NE):
                        nc.tensor.matmul(
                            pq[t],
                            W_sb[:, 1 + b // 2, e, 128 * (b % 2):128 * (b % 2 + 1)],
                            x_sb[:, e, 128 + 512 * t:128 + 512 * (t + 1)],
                            start=(e == 0), stop=(e == NE - 1))
                return pq

            def rope(src, nch, chw, toff, ci, si, add_fn, pool, tag):
                """dst = src*cos + (perm@src)*sin over nch chunks of chw."""
                for ch in range(nch):
                    cs = slice(chw * ch, chw * (ch + 1))
                    ts = slice(toff + chw * ch, toff + chw * (ch + 1))
                    rot = pool.tile([128, 512], f32, tag=tag, name=tag)
                    nc.tensor.matmul(rot[:, 0:chw], perm, src[:, cs],
                                     start=True, stop=True)
                    m1 = rsc.tile([128, 512], bf16, tag="m1", name="m1")
                    nc.vector.tensor_mul(m1[:, 0:chw], src[:, cs],
                                         tabs[:, ci, ts])
                    m2 = rsc.tile([128, 512], bf16, tag="m2", name="m2")
                    nc.vector.tensor_mul(m2[:, 0:chw], rot[:, 0:chw],
                                         tabs[:, si, ts])
                    add_fn(cs, m1[:, 0:chw], m2[:, 0:chw])

            def k_add(cs, m1, m2):
                nc.gpsimd.tensor_add(KR[:, cs], m1, m2)

            def q_drain_rope(b, pq, pool, tag):
                qsb = qsb_pool.tile([128, NQ], bf16, tag="qsb", name="qsb")
                for t in range(2):
                    nc.scalar.activation(
                        out=qsb[:, 512 * t:512 * (t + 1)], in_=pq[t],
                        func=Ident, bias=b_sb[:, 2 + b:3 + b])
                rope(qsb, 2, 512, 128, 2, 3,
                     lambda cs, m1, m2: nc.gpsimd.tensor_add(QG[:, b, cs],
                                                             m1, m2),
                     pool, tag)

            def v_transpose(kbs):
                for kb in kbs:
                    for g in range(KG):
                        pt = psT.tile([128, 64], f32, tag="vt", name="vt")
                        ptb = pt.bitcast(bf16)
                        nc.tensor.transpose(
                            ptb[:, 0:64],
                            Vsb[64 * g:64 * (g + 1), 128 * kb:128 * (kb + 1)],
                            identb[64 * g:64 * (g + 1), 64 * g:64 * (g + 1)])
                        nc.vector.tensor_copy(Vtok[:, kb, g, 0:64],
                                              ptb[:, 0:64])

            pq0 = q_block_mm(0, psQ, "q")
            rope(Ksb, 3, 384, 0, 0, 1, k_add, psR, "rot")
            pq1 = q_block_mm(1, psQ, "q")
            q_drain_rope(0, pq0, psR, "rot")
            v_transpose(range(0, 5))
            pq2 = q_block_mm(2, psQ, "q")
            q_drain_rope(1, pq1, psR, "rot")
            v_transpose(range(5, 9))
            pq3 = q_block_mm(3, psQ, "q")
            q_drain_rope(2, pq2, psR, "rot")
            q_drain_rope(3, pq3, psR, "rot")
            es2.close()

            # ============ phase 3: attention + out-projection + Q2/Q3 =====
            psS = es.enter_context(
                tc.tile_pool(name="psS", bufs=1, space="PSUM"))
            psC = es.enter_context(
                tc.tile_pool(name="psC", bufs=2, space="PSUM"))
            psRr = es.enter_context(
                tc.tile_pool(name="psRr", bufs=2, space="PSUM"))
            flex = es.enter_context(
                tc.tile_pool(name="flex", bufs=2, space="PSUM"))
            pp = es.enter_context(tc.tile_pool(name="pp", bufs=5))
            pcx = es.enter_context(tc.tile_pool(name="pcx", bufs=6))
            prv = es.enter_context(tc.tile_pool(name="prv", bufs=6))
            po = es.enter_context(tc.tile_pool(name="po", bufs=2))

            if DEBUG:
                nc.sync.dma_start(dbgKR, KR)
                nc.sync.dma_start(dbgQG, QG)
                nc.sync.dma_start(dbgVt, Vtok)

            def attn_scores(qb, g):
                """Scores + mask + exp for 4 heads of kv-group g, block qb."""
                ps = psS.tile([128, 2, 512], f32, tag="ps", name="ps")
                qsl = QG[64 * g:64 * (g + 1), :, 128 * qb:128 * (qb + 1)]
                for kb in range(2):
                    mv = 0 if (kb == 0 and qb == 0) else (1 if kb == 0 else 2)
                    nc.tensor.matmul(ps[:, kb, :], masks[:, mv, :], Irep,
                                     start=True, stop=False)
                    nc.tensor.matmul(
                        ps[:, kb, :],
                        KR[64 * g:64 * (g + 1),
                           128 * (qb + kb):128 * (qb + kb + 1)],
                        qsl, start=False, stop=True)
                p = pp.tile([128, 2, 512], bf16, tag="p", name="p")
                nc.scalar.activation(out=p[:, 0, :], in_=ps[:, 0, :],
                                     func=Exp)
                nc.scalar.activation(out=p[:, 1, :], in_=ps[:, 1, :],
                                     func=Exp)
                if DEBUG and qb == 1 and g == 0:
                    nc.sync.dma_start(dbgP, p)
                return ps, p

            def attn_ctx(qb, g, ps, p):
                """ctx for all 4 heads (+denom row 64) into psC; drains."""
                pc = psC.tile([128, 512], f32, tag="pc", name="pc")
                nc.tensor.matmul(pc[0:65, :], unit65,
                                 esink[0:1, 512 * g:512 * (g + 1)],
                                 start=True, stop=False)
                nc.tensor.matmul(pc[0:65, :], Vtok[:, qb, g, :],
                                 p[:, 0, :], start=False, stop=False)
                nc.tensor.matmul(pc[0:65, :], Vtok[:, qb + 1, g, :],
                                 p[:, 1, :], start=False, stop=True)
                if DEBUG and qb == 1 and g == 0:
                    cpy = pcx.tile([128, 512], f32, tag="dbgc", name="dbgc")
                    nc.scalar.activation(out=cpy, in_=pc, func=Ident)
                    nc.sync.dma_start(dbgC, cpy)
                cx = pcx.tile([64, 4, 128], bf16, tag="cx", name="cx")
                nc.vector.tensor_copy(cx, pc[0:64, :])
                dsb = prv.tile([1, 512], bf16, tag="dn", name="dn")
                nc.vector.tensor_copy(dsb, pc[64:65, :])
                return cx, dsb

            def attn_norm(qb, g, cx, dsb):
                """Replicate denom across 64 partitions (PE, bf16), 64-lane
                reciprocal, then scale ctx into ctxT (GpSimd, which may
                shift partitions for the odd heads)."""
                rr = psRr.tile([64, 4, 128], f32, tag="rr", name="rr")
                th, qq = qb // 4, qb % 4
                for hh in range(4):
                    nc.tensor.matmul(rr[:, hh, :], ones64,
                                     dsb[0:1, 128 * hh:128 * (hh + 1)],
                                     start=True, stop=True)
                rds = prv.tile([64, 4, 128], f32, tag="rd", name="rd")
                nc.scalar.activation(out=rds, in_=rr, func=Ident)
                rinv = prv.tile([64, 4, 128], f32, tag="rv", name="rv")
                nc.vector.reciprocal_approx_fast(rinv, rds)
                for hh in range(4):
                    pair, half = 2 * g + hh // 2, hh % 2
                    nc.gpsimd.tensor_mul(
                        ctxT[64 * half:64 * (half + 1), pair, th,
                             128 * qq:128 * (qq + 1)],
                        cx[:, hh, :], rinv[:, hh, :])

            oq = [nc.sync, nc.scalar]

            # out-projection in 4 chunks of 256 q-cols (one per qb-pair),
            # each split into 8 pieces of 2 e-chunks for weaving.
            op_osb = {}

            def op_piece(ci, pi):
                th, pr = ci // 2, ci % 2
                if pi == 0:
                    op_osb[ci] = po.tile([128, NE, 256], bf16,
                                         tag=f"o{ci % 2}", name=f"o{ci % 2}")
                o_sb = op_osb[ci]
                csl = slice(256 * pr, 256 * (pr + 1))
                pso = flex.tile([128, 512], f32, tag="fx", name="fx")
                for el in range(2):
                    e = 2 * pi + el
                    for h4 in range(4):
                        nc.tensor.matmul(pso[:, 256 * el:256 * (el + 1)],
                                         Wo[:, h4, 128 * e:128 * (e + 1)],
                                         ctxT[:, h4, th, csl],
                                         start=(h4 == 0), stop=(h4 == 3))
                for el in range(2):
                    osl = o_sb[:, 2 * pi + el, :]
                    pss = pso[:, 256 * el:256 * (el + 1)]
                    if pi % 2 == 0:
                        nc.scalar.activation(out=osl, in_=pss, func=Ident)
                    else:
                        nc.vector.tensor_copy(osl, pss)
                if pi in (3, 7):
                    eh = slice(8 * (pi // 4), 8 * (pi // 4) + 8)
                    oq[(ci + pi // 4) % 2].dma_start(
                        outT[:, eh, 512 * th + 256 * pr:
                             512 * th + 256 * (pr + 1)], o_sb[:, eh, :])

            # software-pipelined emission: scores(i) | ctx(i-1) | norm(i-2);
            # out-projection pieces woven in as their qb-pair completes.
            groups = [(qb, g) for qb in range(8) for g in range(2)]
            sc_out = {}
            cx_out = {}
            op_queue = []

            def emit_fill(i):
                n = 3 if len(op_queue) > 8 else 2
                for _ in range(n):
                    if not op_queue:
                        return
                    op_piece(*op_queue.pop(0))

            def after_norm(grp):
                qb, g = grp
                if g == 1 and qb % 2 == 1:
                    ci = qb // 2
                    op_queue.extend((ci, pi) for pi in range(8))

            for i, grp in enumerate(groups):
                sc_out[i] = attn_scores(*grp)
                if i >= 1:
                    cx_out[i - 1] = attn_ctx(*groups[i - 1], *sc_out.pop(i - 1))
                if i >= 2:
                    attn_norm(*groups[i - 2], *cx_out.pop(i - 2))
                    after_norm(groups[i - 2])
                emit_fill(i)
            cx_out[15] = attn_ctx(*groups[15], *sc_out.pop(15))
            attn_norm(*groups[14], *cx_out.pop(14))
            after_norm(groups[14])
            emit_fill(15)
            attn_norm(*groups[15], *cx_out.pop(15))
            after_norm(groups[15])
            while op_queue:
                op_piece(*op_queue.pop(0))
            if DEBUG:
                nc.scalar.dma_start(dbgCT, ctxT)

    nc.compile()
    return nc


# ---------------- host-side prep ----------------

def _rope_tables(position_ids, gstart):
    pos = np.zeros(TOK, dtype=np.float32)
    idx = gstart + np.arange(TOK)
    valid = (idx >= 0) & (idx < S)
    pos[valid] = position_ids[0, idx[valid]].astype(np.float32)
    freqs = (1.0 / ROPE_BASE ** (np.arange(0, D, 2, dtype=np.float32) / D)).astype(np.float32)
    wave_len = 2.0 * np.pi / freqs
    low = ORIG_CTX / BETA_FAST
    high = ORIG_CTX / BETA_SLOW
    t = np.clip((wave_len - low) / (high - low), 0.0, 1.0)
    eff = freqs * (1.0 - t) + (freqs / YARN_SCALE) * t
    conc = 0.1 * np.log(np.float32(YARN_SCALE)) + 1.0
    ang = pos[:, None] * eff[None, :] * conc
    sin = np.sin(ang).astype(np.float32).T    # [32, TOK]
    cos = np.cos(ang).astype(np.float32).T
    cosT = np.concatenate([cos, cos], axis=0)  # [64, TOK]
    sinS = np.concatenate([-sin, sin], axis=0)
    cos2 = np.concatenate([cosT, cosT], axis=0)  # [128, TOK]
    sinS2 = np.concatenate([sinS, sinS], axis=0)
    return np.ascontiguousarray(cos2), np.ascontiguousarray(sinS2)


def _build_masks(s):
    """Additive mask matrices, transposed for the PE mask-add:
    M_store[q, kv] = MASKVAL where kv is invalid for q.
    var0: block A for qb==0; var1: block A std; var2: block B."""
    q = np.arange(128)[:, None]
    kv = np.arange(128)[None, :]
    m_a = np.where(kv <= q, MASKVAL, 0.0).astype(np.float32)   # A: valid kv>q
    m_b = np.where(kv > q, MASKVAL, 0.0).astype(np.float32)    # B: valid kv<=q
    m_a0 = np.full((128, 128), MASKVAL, dtype=np.float32) if s == 0 else m_a
    return np.stack([m_a0, m_a, m_b], axis=1)                  # [128, 3, 128]


def _perm_matrix():
    """lhsT for rotate-half: out[p] = src[p xor 32] within each 64-half."""
    P = np.zeros((128, 128), dtype=np.float32)
    for m in range(128):
        half = (m // 64) * 64
        pi = half + ((m - half) + 32) % 64
        P[pi, m] = 1.0
    return P


def _prep_core(c, position_ids, Wqkv, bqkv, Wout, sinks, xT_full):
    s, h = c // 4, c % 4
    gstart = 1024 * s - 128
    xTc = np.zeros((E, TOK), dtype=np.float32)
    lo = max(0, gstart)
    xTc[:, lo - gstart:TOK] = xT_full[:, lo:gstart + TOK]
    # feature rows: K (2 groups), V (2 groups), Q blocks b = heads (b, 4+b)
    krows = np.arange(H * D + 128 * h, H * D + 128 * h + 128)
    vrows = np.arange((H + G) * D + 128 * h, (H + G) * D + 128 * h + 128)
    qrows = []
    for b in range(4):
        for l in (b, 4 + b):
            g_head = 8 * h + l
            qrows.append(np.arange(64 * g_head, 64 * g_head + 64))
    qrows = np.concatenate(qrows)
    rows = np.concatenate([krows, vrows, qrows])
    WqkvTc = np.ascontiguousarray(Wqkv[rows].T)
    bq = bqkv[rows].reshape(FTOT // 128, 128).T
    WoutTc = np.ascontiguousarray(Wout[:, 512 * h:512 * h + 512].T)
    cos2, sinS2 = _rope_tables(position_ids, gstart)
    masks = _build_masks(s)
    # esink: [1, 8*128], score-column head order [4g, 4g+2, 4g+1, 4g+3]
    es_l = np.exp(sinks[0, 8 * h:8 * h + 8, 0, 0]).astype(np.float32)
    esink = np.repeat(es_l, 128)[None, :]
    import ml_dtypes
    bf = ml_dtypes.bfloat16
    xP = xTc.reshape(NE, 128, TOK).transpose(1, 0, 2)
    wP = (WqkvTc.reshape(NE, 128, FTOT).transpose(1, 0, 2)
          .reshape(128, NE, 3, 256).transpose(0, 2, 1, 3))
    woP = WoutTc.reshape(4, 128, E).transpose(1, 0, 2)
    tabs = np.stack([cos2, sinS2, SCALE * cos2, SCALE * sinS2], axis=1)
    return {
        "xT": np.ascontiguousarray(xP.astype(bf)),
        "wqkvT": np.ascontiguousarray(wP.astype(bf)),
        "bqkvT": np.ascontiguousarray(bq.astype(np.float32)),
        "woutT": np.ascontiguousarray(woP.astype(bf)),
        "tabs": np.ascontiguousarray(tabs.astype(bf)),
        "masks": np.ascontiguousarray(masks.astype(bf)),
        "esink": np.ascontiguousarray(esink.astype(bf)),
        "perm": np.ascontiguousarray(_perm_matrix().astype(bf)),
    }


def _prep_all(inputs):
    x = np.asarray(inputs["x"], dtype=np.float32)
    position_ids = np.asarray(inputs["position_ids"])
    Wqkv = np.asarray(inputs["Wqkv"], dtype=np.float32)
    bqkv = np.asarray(inputs["bqkv"], dtype=np.float32)
    Wout = np.asarray(inputs["Wout"], dtype=np.float32)
    sinks = np.asarray(inputs["sinks"], dtype=np.float32)
    xT_full = np.ascontiguousarray(x[0].T)
    return [
        _prep_core(c, position_ids, Wqkv, bqkv, Wout, sinks, xT_full)
        for c in range(NCORES)
    ]


def kernel(x, position_ids, attn_mask, Wqkv, bqkv, Wout, bout, sinks):
    global _compiled
    from concourse.bass_utils import run_bass_kernel_spmd

    bout = np.asarray(bout, dtype=np.float32)

    if _compiled is None:
        _compiled = _build_bass()
    nc = _compiled

    in_maps = _prep_all({
        "x": x, "position_ids": position_ids,
        "Wqkv": Wqkv, "bqkv": bqkv, "Wout": Wout, "sinks": sinks,
    })
    res = run_bass_kernel_spmd(nc, in_maps, list(range(NCORES)))

    out = np.empty((S, E), dtype=np.float32)
    for s in range(2):
        acc = res.results[4 * s]["outT"].astype(np.float32)
        for h in range(1, 4):
            acc = acc + res.results[4 * s + h]["outT"].astype(np.float32)
        out[1024 * s:1024 * (s + 1)] = acc.transpose(1, 0, 2).reshape(E, NQ).T
    out += bout[None, :]
    return out[None]
